# revision 9
# baseline (speedup 1.0000x reference)
"""EGNN-style message-passing layer on 8 Trainium2 NeuronCores.

Strategy (edge-parallel + node-partitioned output):
  * Host sorts edges by destination node and partitions them across the 8
    cores by destination block (each core owns a contiguous slice of
    n_nodes/8 nodes = bpc blocks of 128 nodes).  Every block's edge list is
    padded to a uniform B_max so all cores run the identical program.
  * Node features h (bf16) and coords x are packed into 512-byte rows of a
    replicated table; each core transpose-gathers h[src]/h[dst] (and x)
    straight into feature-major SBUF tiles via SWDGE dma_gather (512
    tokens/call — the ucode crashes above that; src/dst gathers use separate
    SWDGE queues so descriptor generation runs on different Q7 core pairs).
  * Per-edge MLPs run as bf16 matmuls with f32 PSUM accumulation:
      hidden = W1s.T@h_src + W1d.T@h_dst + V.T@silu(We1.T*dist+be1),
      silu(hidden+b1') -> second stage (edge-major) -> [m | w] per edge.
  * Coordinate update: PE-transpose of (x_src-x_dst) to edge-major, then
    normalize and scale by w on DVE/ACT.
  * Aggregation (segment sum): one-hot S matrix per 128-edge subtile
    (iota==dst_local compare) and a PE matmul S.T @ [m|cu] accumulated in
    PSUM across each block's subtiles.  No collectives; each core writes its
    own node slice.
"""

import numpy as np
import ml_dtypes

bf16 = ml_dtypes.bfloat16

N_CORES = 8
BLOCK = 128  # nodes per aggregation block
SUB = 128  # edges per subtile
SUP = 512  # edges per supertile == edges per gather call
NODE_DIM = 128
EDGE_DIM = 32
HID2 = 512  # combined node+coord hidden
ELEM = 128  # gather row elems (bf16): h only -> 256B rows

_programs = {}


def _build_program(n_nodes, s_tot, bpc):
    """Build + compile the per-core bass program."""
    import concourse.bacc as bacc
    import concourse.tile as tile
    import concourse.mybir as mybir

    F32 = mybir.dt.float32
    BF = mybir.dt.bfloat16
    I16 = mybir.dt.int16
    Silu = mybir.ActivationFunctionType.Silu
    Sqrt = mybir.ActivationFunctionType.Sqrt
    Square = mybir.ActivationFunctionType.Square
    Alu = mybir.AluOpType

    spb = (s_tot // bpc) // SUB  # subtiles per block
    n_sub = s_tot // SUB
    n_sup = s_tot // SUP
    dch = 8192 if s_tot % 8192 == 0 else SUP  # dist stream chunk

    nc = bacc.Bacc("TRN2", target_bir_lowering=False, debug=False,
                   num_devices=N_CORES, num_swdge_queues=2)

    # --- DRAM tensors ---
    hx = nc.dram_tensor("hx", [n_nodes, ELEM], BF, kind="ExternalInput")
    sidx = nc.dram_tensor("sidx", [128, s_tot // 16], I16, kind="ExternalInput")
    didx = nc.dram_tensor("didx", [128, s_tot // 16], I16, kind="ExternalInput")
    dstloc = nc.dram_tensor("dstloc", [128, n_sub], F32, kind="ExternalInput")
    dist = nc.dram_tensor("dist", [1, s_tot], BF, kind="ExternalInput")
    hres = nc.dram_tensor("hres", [128, bpc, NODE_DIM], F32, kind="ExternalInput")
    xres = nc.dram_tensor("xres", [128, bpc, 3], F32, kind="ExternalInput")
    w1s = nc.dram_tensor("w1s", [128, HID2], BF, kind="ExternalInput")
    w1d = nc.dram_tensor("w1d", [128, HID2], BF, kind="ExternalInput")
    vw = nc.dram_tensor("vw", [EDGE_DIM, HID2], BF, kind="ExternalInput")
    we1 = nc.dram_tensor("we1", [1, EDGE_DIM], BF, kind="ExternalInput")
    be1 = nc.dram_tensor("be1", [EDGE_DIM, 1], F32, kind="ExternalInput")
    b1 = nc.dram_tensor("b1", [128, 4], F32, kind="ExternalInput")
    rw = nc.dram_tensor("rw", [128, 4, 132], BF, kind="ExternalInput")
    bn2rep = nc.dram_tensor("bn2rep", [128, 132], F32, kind="ExternalInput")
    udir = nc.dram_tensor("udir", [128, n_sub, 3], F32, kind="ExternalInput")
    hout = nc.dram_tensor("hout", [128, bpc, NODE_DIM], F32, kind="ExternalOutput")
    xout = nc.dram_tensor("xout", [128, bpc, 3], F32, kind="ExternalOutput")

    with tile.TileContext(nc) as tc:
        with (
            tc.tile_pool(name="const", bufs=1) as cp,
            tc.tile_pool(name="gather", bufs=3) as gp,
            tc.tile_pool(name="diststr", bufs=2) as dp,
            tc.tile_pool(name="work", bufs=2) as wp,
            tc.tile_pool(name="small", bufs=3) as sp,
            tc.tile_pool(name="ph", bufs=1, space="PSUM") as pool_ph,
            tc.tile_pool(name="pea", bufs=1, space="PSUM") as pool_ea,
            tc.tile_pool(name="p23", bufs=2, space="PSUM") as pool_p23,
            tc.tile_pool(name="pblk", bufs=1, space="PSUM") as pool_blk,
        ):
            # --- constants ---
            sidx_t = cp.tile([128, s_tot // 16], I16)
            nc.sync.dma_start(sidx_t[:], sidx[:])
            didx_t = cp.tile([128, s_tot // 16], I16)
            nc.sync.dma_start(didx_t[:], didx[:])
            dstloc_t = cp.tile([128, n_sub], F32)
            nc.sync.dma_start(dstloc_t[:], dstloc[:])
            hres_t = cp.tile([128, bpc, NODE_DIM], F32)
            nc.sync.dma_start(hres_t[:], hres[:])
            xres_t = cp.tile([128, bpc, 3], F32)
            nc.sync.dma_start(xres_t[:], xres[:])
            w1s_t = cp.tile([128, HID2], BF)
            nc.sync.dma_start(w1s_t[:], w1s[:])
            w1d_t = cp.tile([128, HID2], BF)
            nc.sync.dma_start(w1d_t[:], w1d[:])
            vw_t = cp.tile([EDGE_DIM, HID2], BF)
            nc.sync.dma_start(vw_t[:], vw[:])
            we1_t = cp.tile([1, EDGE_DIM], BF)
            nc.sync.dma_start(we1_t[:], we1[:])
            be1_t = cp.tile([EDGE_DIM, 1], F32)
            nc.sync.dma_start(be1_t[:], be1[:])
            b1_t = cp.tile([128, 4], F32)
            nc.sync.dma_start(b1_t[:], b1[:])
            rw_t = cp.tile([128, 4, 132], BF)
            nc.sync.dma_start(rw_t[:], rw[:])
            bn2_t = cp.tile([128, 132], F32)
            nc.sync.dma_start(bn2_t[:], bn2rep[:])
            udir_t = cp.tile([128, n_sub, 3], F32)
            nc.sync.dma_start(udir_t[:], udir[:])
            iota_t = cp.tile([128, 128], F32)
            nc.gpsimd.iota(iota_t[:], [[1, 128]], channel_multiplier=0,
                           allow_small_or_imprecise_dtypes=True)

            pblk = None
            dist_t = None
            for u in range(n_sup):
                ic = slice(u * (SUP // 16), (u + 1) * (SUP // 16))
                gs = gp.tile([128, 1, SUP], BF, tag="gs")
                nc.gpsimd.dma_gather(gs[:], hx[:], sidx_t[:, ic], SUP, SUP,
                                     ELEM, transpose=True, queue_num=0)
                gd = gp.tile([128, 1, SUP], BF, tag="gd")
                nc.gpsimd.dma_gather(gd[:], hx[:], didx_t[:, ic], SUP, SUP,
                                     ELEM, transpose=True, queue_num=1)
                if u % (dch // SUP) == 0:
                    dist_t = dp.tile([1, dch], BF, tag="dist")
                    d0 = u * SUP
                    nc.sync.dma_start(dist_t[:], dist[:, d0:d0 + dch])
                e0 = (u * SUP) % dch

                # edge mlp first layer: [32, SUP] = We1.T @ dist
                pea = pool_ea.tile([EDGE_DIM, SUP], F32, tag="pea")
                nc.tensor.matmul(pea[:], we1_t[:], dist_t[:, e0:e0 + SUP],
                                 start=True, stop=True)
                sea = wp.tile([EDGE_DIM, SUP], BF, tag="sea")
                nc.scalar.activation(sea[:], pea[:], Silu, bias=be1_t[:])

                # hidden = W1s.T@hs + W1d.T@hd + V.T@sea  (4 chunks of 128)
                ph = pool_ph.tile([128, 4, SUP], F32, tag="ph")
                for j in range(4):
                    jc = slice(j * 128, (j + 1) * 128)
                    nc.tensor.matmul(ph[:, j, :], w1s_t[:, jc], gs[:, 0, :],
                                     start=True, stop=False)
                    nc.tensor.matmul(ph[:, j, :], w1d_t[:, jc], gd[:, 0, :],
                                     start=False, stop=False)
                    nc.tensor.matmul(ph[:, j, :], vw_t[:, jc], sea[:],
                                     start=False, stop=True)
                siluh = wp.tile([128, 4, SUP], BF, tag="siluh")
                for j in range(4):
                    nc.scalar.activation(siluh[:, j, :], ph[:, j, :], Silu,
                                         bias=b1_t[:, j:j + 1])


                for q in range(SUP // SUB):
                    s = u * (SUP // SUB) + q
                    es = slice(q * SUB, (q + 1) * SUB)
                    # 2nd stage (edge-major): psum[:,0:132] accumulates
                    # [m | w | 0]; psum[:,132:135] gets dir transpose.
                    p23 = pool_p23.tile([128, 132], F32, tag="p23")
                    for k in range(4):
                        nc.tensor.matmul(p23[:, 0:132], siluh[:, k, es],
                                         rw_t[:, k, :],
                                         start=(k == 0), stop=(k == 3))
                    # V = [m + bn2 | cu] (bf16) for the agg matmul
                    vt = sp.tile([128, 132], BF, tag="vt")
                    nc.vector.tensor_tensor(vt[:, 0:132], p23[:, 0:132],
                                            bn2_t[:], Alu.add)
                    # cu = w * unit_dir (host-streamed, edge-major)
                    nc.vector.tensor_scalar_mul(vt[:, 128:131],
                                                udir_t[:, s, :],
                                                p23[:, 128:129])

                    # one-hot S for this subtile
                    st = sp.tile([128, 128], BF, tag="st")
                    nc.vector.tensor_scalar(st[:], iota_t[:],
                                            dstloc_t[:, s:s + 1], None,
                                            Alu.is_equal)

                    # aggregate into the block accumulator
                    b, sb = divmod(s, spb)
                    if sb == 0:
                        pblk = pool_blk.tile([128, 131], F32, tag="pblk")
                    nc.tensor.matmul(pblk[:], st[:], vt[:, 0:131],
                                     start=(sb == 0), stop=(sb == spb - 1))
                    if sb == spb - 1:
                        oh = sp.tile([128, NODE_DIM], F32, tag="oh")
                        nc.vector.tensor_tensor(oh[:], pblk[:, 0:128],
                                                hres_t[:, b, :], Alu.add)
                        nc.sync.dma_start(hout[:, b, :], oh[:])
                        ox = sp.tile([128, 3], F32, tag="ox")
                        nc.vector.tensor_tensor(ox[:], pblk[:, 128:131],
                                                xres_t[:, b, :], Alu.add)
                        nc.sync.dma_start(xout[:, b, :], ox[:])

    nc.compile()
    return nc


def _prep(h, x, edge_index, edge_dist, We1, be1, We2, be2, Wn1, bn1, Wn2, bn2,
          Wc1, bc1, Wc2):
    """Host-side sharding: sort edges by destination block, pad uniformly."""
    n_nodes = h.shape[0]
    n_blocks = n_nodes // BLOCK
    bpc = n_blocks // N_CORES

    src = np.asarray(edge_index[0], np.int64)
    dst = np.asarray(edge_index[1], np.int64)
    dist = np.asarray(edge_dist, np.float32)

    order = np.argsort(dst, kind="stable")
    src_s, dst_s, dist_s = src[order], dst[order], dist[order]
    blk = (dst_s // BLOCK).astype(np.int64)
    counts = np.bincount(blk, minlength=n_blocks)
    b_max = max(int(-(-counts.max() // SUB) * SUB), SUB)
    s_tot = bpc * b_max
    assert s_tot % SUP == 0

    # padded position of each sorted edge
    block_start = np.zeros(n_blocks, np.int64)
    block_start[1:] = np.cumsum(counts)[:-1]
    rank = np.arange(len(dst_s)) - block_start[blk]
    pad_pos = blk * b_max + rank  # global padded position across all cores

    tot = n_blocks * b_max
    src_p = np.zeros(tot, np.int64)
    dst_p = np.zeros(tot, np.int64)
    dist_p = np.zeros(tot, np.float32)
    dloc_p = np.full(tot, -1.0, np.float32)
    src_p[pad_pos] = src_s
    dst_p[pad_pos] = dst_s
    dist_p[pad_pos] = dist_s
    dloc_p[pad_pos] = (dst_s % BLOCK).astype(np.float32)

    def wrap_idx(a):  # [s_tot] int16 -> [128, s_tot//16] per-SUP-call wrapped
        cols = [a[c * SUP:(c + 1) * SUP].reshape(SUP // 16, 16).T
                for c in range(s_tot // SUP)]
        return np.tile(np.concatenate(cols, axis=1), (8, 1)).astype(np.int16)

    # gather table: h only (256B rows)
    hxp = h.astype(bf16)

    # host unit_dir per padded edge (pads have src=dst=0 -> dir=0 -> u=0)
    dv = x[src_p] - x[dst_p]
    dl = np.maximum(np.sqrt((dv * dv).sum(-1, keepdims=True)), 1e-8)
    u = (dv / dl).astype(np.float32)  # [tot, 3]

    per_core = []
    for c in range(N_CORES):
        sl = slice(c * s_tot, (c + 1) * s_tot)
        per_core.append({
            "sidx": wrap_idx(src_p[sl].astype(np.int16)),
            "didx": wrap_idx(dst_p[sl].astype(np.int16)),
            "dstloc": dloc_p[sl].reshape(s_tot // SUB, SUB).T.astype(np.float32).copy(),
            "dist": dist_p[sl].reshape(1, s_tot).astype(bf16),
            "udir": u[sl].reshape(s_tot // SUB, SUB, 3).transpose(1, 0, 2).copy(),
        })


    # weights
    W1 = np.concatenate([Wn1, Wc1], axis=1)  # [288, 512]
    W1e = W1[2 * NODE_DIM:]  # [32, 512]
    V = (We2 @ W1e).astype(bf16)  # [32, 512]
    b1p = (np.concatenate([bn1, bc1]) + be2 @ W1e).astype(np.float32)  # [512]
    rw = np.zeros((4, 128, 132), np.float32)
    for k in range(2):
        rw[k, :, 0:128] = Wn2[k * 128:(k + 1) * 128]
    for k in range(2, 4):
        rw[k, :, 128:129] = Wc2[(k - 2) * 128:(k - 1) * 128]
    rw = rw.transpose(1, 0, 2).copy().astype(bf16)  # [128, 4, 132]
    bn2rep = np.zeros((128, 132), np.float32)
    bn2rep[:, 0:128] = bn2[None, :]

    shared = {
        "hx": hxp,
        "w1s": W1[0:128].astype(bf16),
        "w1d": W1[128:256].astype(bf16),
        "vw": V,
        "we1": We1.astype(bf16),
        "be1": be1.reshape(EDGE_DIM, 1).astype(np.float32),
        "b1": b1p.reshape(4, 128).T.copy(),
        "rw": rw,
        "bn2rep": bn2rep,
    }
    for c in range(N_CORES):
        h_sl = h[c * bpc * BLOCK:(c + 1) * bpc * BLOCK]
        x_sl = x[c * bpc * BLOCK:(c + 1) * bpc * BLOCK]
        per_core[c]["hres"] = h_sl.reshape(bpc, BLOCK, NODE_DIM).transpose(1, 0, 2).copy()
        per_core[c]["xres"] = x_sl.reshape(bpc, BLOCK, 3).transpose(1, 0, 2).copy()
        per_core[c].update(shared)

    return per_core, (n_nodes, s_tot, bpc)


def _run(inputs, trace=False, trace_kwargs=None, trace_cores=None):
    from concourse import bass_utils

    args = {k: np.asarray(v) for k, v in inputs.items()}
    h = args["h"].astype(np.float32)
    x = args["x"].astype(np.float32)
    per_core, key = _prep(
        h, x, args["edge_index"], args["edge_dist"],
        args["We1"].astype(np.float32), args["be1"].astype(np.float32),
        args["We2"].astype(np.float32), args["be2"].astype(np.float32),
        args["Wn1"].astype(np.float32), args["bn1"].astype(np.float32),
        args["Wn2"].astype(np.float32), args["bn2"].astype(np.float32),
        args["Wc1"].astype(np.float32), args["bc1"].astype(np.float32),
        args["Wc2"].astype(np.float32))

    if key not in _programs:
        _programs[key] = _build_program(*key)
    nc = _programs[key]

    res = bass_utils.run_bass_kernel_spmd(
        nc, per_core, core_ids=list(range(N_CORES)), trace=trace,
        trace_kwargs=trace_kwargs or {}, trace_cores=trace_cores)

    n_nodes, _, bpc = key
    h_out = np.empty((n_nodes, NODE_DIM), np.float32)
    x_out = np.empty((n_nodes, 3), np.float32)
    for c in range(N_CORES):
        sl = slice(c * bpc * BLOCK, (c + 1) * bpc * BLOCK)
        h_out[sl] = res.results[c]["hout"].transpose(1, 0, 2).reshape(-1, NODE_DIM)
        x_out[sl] = res.results[c]["xout"].transpose(1, 0, 2).reshape(-1, 3)
    return (h_out, x_out), res


def kernel(**inputs):
    out, _ = _run(inputs)
    return out
